# revision 25
# baseline (speedup 1.0000x reference)
"""TRN2 Bass kernel: 8-layer Chambolle-Pock MRI reconstruction on 8 NeuronCores.

Sharding: frames (8/core) for FFTs + elementwise; k-space rows (8192 px/core)
for the low-rank prox (Gram trick + matmul-only spectral filter, no eigensolver).
Cross-core per layer: AllToAll (y1 frame->pixel), AllToAll (x1k V pixel->frame),
AllReduce (64x64 Gram), 2 small AllGathers (temporal halo). Mostly bf16 storage
with fp32 PSUM accumulation.

The spectral-filter chain works on the real symmetric 2x2-block embedding
S(A) = [[Ar, -Ai], [Ai, Ar]] of the Hermitian 64x64 matrices (all chain
matrices are polynomials of the Gram matrix, hence Hermitian + commuting):
every complex product is a single 128x128x128 matmul.
"""
import numpy as np

import concourse.bass as bass
import concourse.bacc as bacc
import concourse.mybir as mybir
import concourse.tile as tile
from concourse import bass_utils

F32 = mybir.dt.float32
BF16 = mybir.dt.bfloat16
U32 = mybir.dt.uint32
AF = mybir.ActivationFunctionType
ALU = mybir.AluOpType

NCORE = 8
NF = 64
NFL = 8
N = 256
NLAYERS = 8
SIGMA = float(np.float32(1.0 / np.sqrt(8.0)))
TAU = SIGMA
C1 = float(np.float32(1.0 / (1.0 + SIGMA)))
C2 = C1 * SIGMA
N_SQ = 8          # norm-estimation squarings (grouped renorm every 3)
SIGN_COEFFS = [(3.4445, -4.7750, 2.0315)] * 5 + [(1.875, -1.25, 0.375)] * 2
Q_DEG = 9
RG = [list(range(NCORE))]


def _fit_q(deg=Q_DEG, lo=0.068, hi=1.05, npts=6000):
    xs_in = np.linspace(lo, hi, npts)
    xs_out = np.linspace(0, 0.05, 300)

    def cheb(x, d, b=hi):
        t = (2 * x - b) / b
        V = np.zeros((len(x), d + 1))
        V[:, 0] = 1
        if d >= 1:
            V[:, 1] = t
        for k in range(2, d + 1):
            V[:, k] = 2 * t * V[:, k - 1] - V[:, k - 2]
        return V

    Vi = cheb(xs_in, deg)
    Vo = cheb(xs_out, deg)
    A = np.vstack([np.sqrt(xs_in)[:, None] * Vi, 1e-5 * Vo])
    y = np.concatenate([np.ones(npts), np.zeros(len(xs_out))])
    coef, *_ = np.linalg.lstsq(A, y, rcond=None)
    return [float(c) for c in coef], hi


Q_COEF, Q_HI = _fit_q()


def _dft():
    k = np.arange(N)
    W = np.exp(-2j * np.pi * np.outer(k, k) / N)
    return W.real.astype(np.float32), W.imag.astype(np.float32)


def _chunk(a):
    return np.ascontiguousarray(np.stack([a[0:128], a[128:256]], axis=1))


def build(n_layers=NLAYERS, single_core=False):
    nc = bacc.Bacc("TRN2", target_bir_lowering=False, debug=False,
                   num_devices=1 if single_core else NCORE)

    dk_r = nc.dram_tensor("dk_r", [128, 2, NFL, N], F32, kind="ExternalInput")
    dk_i = nc.dram_tensor("dk_i", [128, 2, NFL, N], F32, kind="ExternalInput")
    dps = nc.dram_tensor("dps", [128, 8192], F32, kind="ExternalInput")
    lamS_in = nc.dram_tensor("lamS", [1, NLAYERS], F32, kind="ExternalInput")
    lamL_in = nc.dram_tensor("lamL", [1, NLAYERS], F32, kind="ExternalInput")
    cmeta = nc.dram_tensor("cmeta", [1, 4], F32, kind="ExternalInput")
    cmeta_u = nc.dram_tensor("cmeta_u", [1, 2], U32, kind="ExternalInput")
    out_r = nc.dram_tensor("out_r", [128, 2, NFL, N], F32, kind="ExternalOutput")
    out_i = nc.dram_tensor("out_i", [128, 2, NFL, N], F32, kind="ExternalOutput")

    Fr, Fi = _dft()
    sA = C1 * SIGMA / N
    sI = -TAU / N
    id128 = np.eye(128, dtype=np.float32)
    consts_np = {
        "SA0": (_chunk(np.concatenate([Fr * sA, Fi * sA], 1)), BF16),
        "SA1": (_chunk(np.concatenate([-Fi * sA, Fr * sA], 1)), BF16),
        "SI0": (_chunk(np.concatenate([Fr * sI, -Fi * sI], 1)), BF16),
        "SI1": (_chunk(np.concatenate([Fi * sI, Fr * sI], 1)), BF16),
        "Br": (_chunk(Fr), BF16),
        "Bi": (_chunk(Fi), BF16),
        "Bn": (_chunk(-Fi), BF16),
        "I128b": (id128, BF16),
        "Ic1": (id128 * C1, BF16),
        "I1b": (id128, BF16),
        "ITb": (id128 * TAU, BF16),
        "ITnb": (id128 * (-TAU), BF16),
        "I128f": (id128, F32),
    }
    handles = {}
    for k, (v, dt) in consts_np.items():
        handles[k] = nc.inline_tensor(v.astype(mybir.dt.np(dt)), name="c" + k)

    b1_in = nc.dram_tensor("b1_in", [NCORE, 2, NFL, 32, N], BF16)
    b1_out = nc.dram_tensor("b1_out", [NCORE, 2, NFL, 32, N], BF16)
    b2_in = nc.dram_tensor("b2_in", [NCORE, 2, NFL, 32, N], BF16)
    b2_out = nc.dram_tensor("b2_out", [NCORE, 2, NFL, 32, N], BF16)
    g_in = nc.dram_tensor("g_in", [64, 192], F32)
    g_out = nc.dram_tensor("g_out", [64, 192], F32, addr_space="Shared")
    nrm_in = nc.dram_tensor("nrm_in", [1, 1], F32)
    nrm_out = nc.dram_tensor("nrm_out", [1, 1], F32, addr_space="Shared")
    hu_in = nc.dram_tensor("hu_in", [1, 2, 128, 2, N], BF16)
    hu_out = nc.dram_tensor("hu_out", [NCORE, 2, 128, 2, N], BF16, addr_space="Shared")
    hy_in = nc.dram_tensor("hy_in", [1, 2, 128, 2, N], BF16)
    hy_out = nc.dram_tensor("hy_out", [NCORE, 2, 128, 2, N], BF16, addr_space="Shared")

    with tile.TileContext(nc) as tc:
        import contextlib
        stack = contextlib.ExitStack()

        def pool(name, bufs, space="SBUF"):
            return stack.enter_context(tc.tile_pool(name=name, bufs=bufs, space=space))

        cp = pool("cp", 1)
        sbp = pool("sbp", 1)
        stp = pool("stp", 1)
        rp = pool("rp", 2)
        bigp = pool("bigp", 1)
        tkp = pool("tkp", 2)
        wp = pool("wp", 1)
        psp = pool("psp", 4, space="PSUM")
        psm = pool("psm", 1, space="PSUM")
        psw = pool("psw", 3, space="PSUM")

        def coll(kind, op, replica_groups, ins, outs):
            if single_core:
                nc.sync.dma_start(out=outs[0], in_=ins[0]) if ins[0].size() == outs[0].size() \
                    else nc.sync.dma_start(out=outs[0][0:1], in_=ins[0][0:1])
            else:
                nc.gpsimd.collective_compute(kind, op, replica_groups=replica_groups,
                                             ins=ins, outs=outs)

        C = {}
        for k, (v, dt) in consts_np.items():
            t = cp.tile(list(v.shape), dt, name="k" + k, tag="k" + k)
            if v.ndim == 2:
                nc.sync.dma_start(out=t[:, :], in_=handles[k][:, :])
            else:
                nc.sync.dma_start(out=t[:, :, :], in_=handles[k][:, :, :])
            C[k] = t

        y1r = sbp.tile([128, 2, NFL, N], BF16, name="y1r", tag="y1r")
        y1i = sbp.tile([128, 2, NFL, N], BF16, name="y1i", tag="y1i")
        u2r = sbp.tile([128, 2, NFL, N], BF16, name="u2r", tag="u2r")
        u2i = sbp.tile([128, 2, NFL, N], BF16, name="u2i", tag="u2i")
        y2r = sbp.tile([128, 2, NFL, N], BF16, name="y2r", tag="y2r")
        y2i = sbp.tile([128, 2, NFL, N], BF16, name="y2i", tag="y2i")
        x2r = sbp.tile([128, 2, NFL, N], BF16, name="x2r", tag="x2r")
        x2i = sbp.tile([128, 2, NFL, N], BF16, name="x2i", tag="x2i")
        # resident V = C2*u1k (frames x pixels of this core)
        vkr = sbp.tile([128, 2, NFL, N], BF16, name="vkr", tag="vkr")
        vki = sbp.tile([128, 2, NFL, N], BF16, name="vki", tag="vki")
        # pixel side, stacked: rows 0:64 = Re(frame f), rows 64:128 = Im(frame f)
        xps = sbp.tile([128, 8192], BF16, name="xps", tag="xps")
        y1ps = sbp.tile([128, 8192], BF16, name="y1ps", tag="y1ps")
        hu_r = sbp.tile([128, 2, N], BF16, name="hu_r", tag="hu_r")
        hu_i = sbp.tile([128, 2, N], BF16, name="hu_i", tag="hu_i")
        hy_r = sbp.tile([128, 2, N], BF16, name="hy_r", tag="hy_r")
        hy_i = sbp.tile([128, 2, N], BF16, name="hy_i", tag="hy_i")
        IT7 = sbp.tile([128, 128], BF16, name="IT7", tag="IT7")
        ITn0 = sbp.tile([128, 128], BF16, name="ITn0", tag="ITn0")
        sc = sbp.tile([128, 8], F32, name="sc", tag="sc")

        def bc(col):
            return sc[:, col:col + 1]

        # ---------------- init ----------------
        dsr = bigp.tile([128, 2, NFL, N], F32, name="dsr", tag="bigs", bufs=2)
        dsi = bigp.tile([128, 2, NFL, N], F32, name="dsi", tag="bigs", bufs=2)
        nc.sync.dma_start(out=dsr[:, :, :, :], in_=dk_r[:, :, :, :])
        nc.sync.dma_start(out=dsi[:, :, :, :], in_=dk_i[:, :, :, :])
        acc = stp.tile([128, 2], F32, name="acc", tag="acc")
        sq = bigp.tile([128, 2048], F32, name="sq", tag="big", bufs=1)
        nc.scalar.activation(sq[:, :], dsr[:, :, 0:4, :], AF.Square, accum_out=acc[:, 0:1])
        sq_b = bigp.tile([128, 2048], F32, name="sq_b", tag="big", bufs=1)
        nc.scalar.activation(sq_b[:, :], dsr[:, :, 4:8, :], AF.Square, accum_out=acc[:, 1:2])
        acs = stp.tile([128, 2], F32, name="acs", tag="acs")
        nc.vector.tensor_add(acs[:, 0:1], acc[:, 0:1], acc[:, 1:2])
        sq_c = bigp.tile([128, 2048], F32, name="sq_c", tag="big", bufs=1)
        nc.scalar.activation(sq_c[:, :], dsi[:, :, 0:4, :], AF.Square, accum_out=acc[:, 0:1])
        sq_d = bigp.tile([128, 2048], F32, name="sq_d", tag="big", bufs=1)
        nc.scalar.activation(sq_d[:, :], dsi[:, :, 4:8, :], AF.Square, accum_out=acc[:, 1:2])
        nc.vector.tensor_add(acs[:, 1:2], acc[:, 0:1], acc[:, 1:2])
        nc.vector.tensor_add(acs[:, 0:1], acs[:, 0:1], acs[:, 1:2])
        on1 = stp.tile([128, 1], F32, name="on1", tag="on1")
        nc.vector.memset(on1[:, :], 1.0)
        onr = stp.tile([1, 128], F32, name="onr", tag="onr")
        nc.vector.memset(onr[0:1, :], 1.0)

        ps0 = psw.tile([128, 8], F32, name="ps0", tag="pw")
        nc.tensor.matmul(ps0[0:1, 0:1], acs[:, 0:1], on1[:, :], start=True, stop=True)
        nrm_st = stp.tile([1, 1], F32, name="nrm_st", tag="nrm")
        nc.vector.tensor_copy(nrm_st[0:1, :], ps0[0:1, 0:1])
        nc.sync.dma_start(out=nrm_in[:, :], in_=nrm_st[0:1, :])
        coll("AllReduce", ALU.add, RG, [nrm_in.ap()], [nrm_out.ap()])
        nc.sync.dma_start(out=nrm_st[0:1, :], in_=nrm_out[:, :])
        rec = stp.tile([1, 1], F32, name="rec", tag="rec")
        nc.vector.reciprocal(rec[0:1, :], nrm_st[0:1, :])
        nc.scalar.activation(rec[0:1, :], rec[0:1, :], AF.Sqrt)
        ps1 = psw.tile([128, 8], F32, name="ps1", tag="pw")
        nc.tensor.matmul(ps1[:, 0:1], onr[0:1, :], rec[0:1, :], start=True, stop=True)
        nc.vector.tensor_copy(bc(0), ps1[:, 0:1])

        nc.vector.tensor_scalar_mul(dsr[:, :, :, :], dsr[:, :, :, :], bc(0))
        nc.vector.tensor_scalar_mul(dsi[:, :, :, :], dsi[:, :, :, :], bc(0))
        # resident tile: V0 = C2 * d-hat
        nc.vector.tensor_scalar_mul(vkr[:, :, :, :], dsr[:, :, :, :], C2)
        nc.vector.tensor_scalar_mul(vki[:, :, :, :], dsi[:, :, :, :], C2)
        nc.gpsimd.dma_start(out=y1ps[:, :], in_=dps[:, :])
        nc.vector.tensor_scalar_mul(xps[:, :], y1ps[:, :], bc(0))

        cm = stp.tile([1, 4], F32, name="cm", tag="cm")
        nc.sync.dma_start(out=cm[0:1, :], in_=cmeta[:, :])
        cmu = stp.tile([1, 2], U32, name="cmu", tag="cmu")
        nc.sync.dma_start(out=cmu[0:1, :], in_=cmeta_u[:, :])
        ps2 = psw.tile([128, 8], F32, name="ps2", tag="pw")
        nc.tensor.matmul(ps2[:, 0:2], onr[0:1, :], cm[0:1, 2:4], start=True, stop=True)
        nc.vector.tensor_copy(bc(1), ps2[:, 0:1])
        nc.vector.tensor_copy(bc(2), ps2[:, 1:2])
        nc.vector.tensor_scalar_mul(bc(3), ps2[:, 1:2], SIGMA)
        nc.vector.tensor_scalar_mul(IT7[:, :], C["ITb"][:, :], bc(2))
        nc.vector.tensor_scalar_mul(ITn0[:, :], C["ITnb"][:, :], bc(1))

        lam_t = stp.tile([1, 2 * NLAYERS], F32, name="lam_t", tag="lam")
        nc.sync.dma_start(out=lam_t[0:1, 0:NLAYERS], in_=lamS_in[:, :])
        nc.sync.dma_start(out=lam_t[0:1, NLAYERS:2 * NLAYERS], in_=lamL_in[:, :])
        nc.vector.tensor_relu(lam_t[0:1, :], lam_t[0:1, :])

        def fwd_stageA(dst, pr_t, pi_t, s0, s1, li, frames=None):
            for f in (range(NFL) if frames is None else frames):
                for sl in range(2):
                    ps = psp.tile([128, 512], F32, name=f"pA{li}_{f}_{sl}", tag="ps")
                    nc.tensor.matmul(ps[:, :], pr_t[:, 0, f, sl * 128:(sl + 1) * 128],
                                     s0[:, 0, :], start=True, stop=False)
                    nc.tensor.matmul(ps[:, :], pi_t[:, 0, f, sl * 128:(sl + 1) * 128],
                                     s1[:, 0, :], start=False, stop=False)
                    nc.tensor.matmul(ps[:, :], pr_t[:, 1, f, sl * 128:(sl + 1) * 128],
                                     s0[:, 1, :], start=False, stop=False)
                    nc.tensor.matmul(ps[:, :], pi_t[:, 1, f, sl * 128:(sl + 1) * 128],
                                     s1[:, 1, :], start=False, stop=True)
                    nc.scalar.copy(out=dst[:, sl, f, :], in_=ps[:, :])

        def stageB(ps_r, ps_i, Yt, m, fp, inv, stop=False):
            for c in range(2):
                wr = C["Br"][:, c, m * 128:(m + 1) * 128]
                wi = C["Bi"][:, c, m * 128:(m + 1) * 128]
                wn = C["Bn"][:, c, m * 128:(m + 1) * 128]
                rr = Yt[:, c, 2 * fp:2 * fp + 2, 0:256]
                ri = Yt[:, c, 2 * fp:2 * fp + 2, 256:512]
                la = (c == 1) and stop
                if not inv:
                    nc.tensor.matmul(ps_r, wr, rr, start=(c == 0), stop=False)
                    nc.tensor.matmul(ps_r, wn, ri, start=False, stop=la)
                    nc.tensor.matmul(ps_i, wi, rr, start=(c == 0), stop=False)
                    nc.tensor.matmul(ps_i, wr, ri, start=False, stop=la)
                else:
                    nc.tensor.matmul(ps_r, wr, rr, start=(c == 0), stop=False)
                    nc.tensor.matmul(ps_r, wi, ri, start=False, stop=la)
                    nc.tensor.matmul(ps_i, wn, rr, start=(c == 0), stop=False)
                    nc.tensor.matmul(ps_i, wr, ri, start=False, stop=la)

        # startpoint: x2 = u2 = ifft2(d-hat) via bf16 copy of d-hat
        nc.vector.tensor_copy(u2r[:, :, :, :], dsr[:, :, :, :])
        nc.vector.tensor_copy(u2i[:, :, :, :], dsi[:, :, :, :])
        Qt0 = bigp.tile([128, 2, NFL, 512], BF16, name="Qt0", tag="big", bufs=1)
        fwd_stageA(Qt0, u2r, u2i, C["SI0"], C["SI1"], "ini")
        for m in range(2):
            for fp in range(4):
                pr = psp.tile([128, 512], F32, name=f"pi0r{m}{fp}", tag="ps")
                pi = psp.tile([128, 512], F32, name=f"pi0i{m}{fp}", tag="ps")
                stageB(pr[:, :], pi[:, :], Qt0, m, fp, inv=True, stop=True)
                sl2 = (slice(None), m, slice(2 * fp, 2 * fp + 2), slice(None))
                nc.scalar.activation(x2r[sl2], pr[:, :], AF.Copy, scale=-1.0 / TAU)
                nc.scalar.activation(x2i[sl2], pi[:, :], AF.Copy, scale=-1.0 / TAU)
                nc.vector.tensor_scalar_mul(u2r[sl2], pr[:, :], -1.0 / TAU)
                nc.vector.tensor_scalar_mul(u2i[sl2], pi[:, :], -1.0 / TAU)
        for t in (y1r, y1i, y2r, y2i):
            nc.vector.memset(t[:, :, :, :], 0.0)

        eng = nc.sync
        r_up = eng.alloc_register("r_up")
        eng.reg_load(r_up, cmu[0:1, 0:1])
        idx_up = eng.snap(r_up, donate=True, min_val=0, max_val=NCORE - 1)
        r_dn = eng.alloc_register("r_dn")
        eng.reg_load(r_dn, cmu[0:1, 1:2])
        idx_dn = eng.snap(r_dn, donate=True, min_val=0, max_val=NCORE - 1)

        def push_u2_halo():
            nc.sync.dma_start(out=hu_in[0, 0, :, :, :], in_=u2r[:, :, 0, :])
            nc.sync.dma_start(out=hu_in[0, 1, :, :, :], in_=u2i[:, :, 0, :])
            coll("AllGather", ALU.bypass, RG, [hu_in.ap()], [hu_out.ap()])
            nc.sync.dma_start(
                out=hu_r[:, :, :],
                in_=hu_out[bass.ds(idx_up, 1), 0, :, :, :].rearrange("o p c y -> (o p) c y"))
            nc.sync.dma_start(
                out=hu_i[:, :, :],
                in_=hu_out[bass.ds(idx_up, 1), 1, :, :, :].rearrange("o p c y -> (o p) c y"))

        push_u2_halo()

        # -------- S-form chain helpers (symmetric 128x128 fp32) --------
        sm = {}

        def smat(key):
            if key not in sm:
                sm[key] = wp.tile([128, 128], F32, name="sm_" + key, tag="sm_" + key)
            return sm[key]

        # ======================= layers =======================
        for li in range(n_layers):
            last = (li == n_layers - 1)

            # ---------- phase K: fwd fft(u2) + y1 update + A2A#1 ----------
            Yt = bigp.tile([128, 2, NFL, 512], BF16, name=f"Yt{li}", tag="big", bufs=1)
            fwd_stageA(Yt, u2r, u2i, C["SA0"], C["SA1"], f"f{li}")
            for m in range(2):
                for fp in range(4):
                    fsl = slice(2 * fp, 2 * fp + 2)
                    sl2 = (slice(None), m, fsl, slice(None))
                    pr = psp.tile([128, 512], F32, name=f"pk_r{li}{m}{fp}", tag="ps")
                    pi = psp.tile([128, 512], F32, name=f"pk_i{li}{m}{fp}", tag="ps")
                    stageB(pr[:, :], pi[:, :], Yt, m, fp, inv=False)
                    nc.tensor.matmul(pr[:, :], C["Ic1"][:, :], y1r[sl2], start=False, stop=False)
                    nc.tensor.matmul(pr[:, :], C["I1b"][:, :], vkr[sl2], start=False, stop=True)
                    nc.tensor.matmul(pi[:, :], C["Ic1"][:, :], y1i[sl2], start=False, stop=False)
                    nc.tensor.matmul(pi[:, :], C["I1b"][:, :], vki[sl2], start=False, stop=True)
                    nc.vector.scalar_tensor_tensor(
                        y1r[sl2], dsr[sl2], -C2,
                        pr[:, :].rearrange("p (a b) -> p a b", a=2),
                        op0=ALU.mult, op1=ALU.add)
                    nc.vector.scalar_tensor_tensor(
                        y1i[sl2], dsi[sl2], -C2,
                        pi[:, :].rearrange("p (a b) -> p a b", a=2),
                        op0=ALU.mult, op1=ALU.add)
            for q in range(4):
                for m in range(2):
                    d = 4 * m + q
                    nc.sync.dma_start(
                        out=b1_in[d, 0, :, :, :].rearrange("f k y -> k f y"),
                        in_=y1r[32 * q:32 * (q + 1), m, :, :])
                    nc.sync.dma_start(
                        out=b1_in[d, 1, :, :, :].rearrange("f k y -> k f y"),
                        in_=y1i[32 * q:32 * (q + 1), m, :, :])
            coll("AllToAll", ALU.bypass, RG, [b1_in.ap()], [b1_out.ap()])

            # ---------- image branch: y2 update ----------
            lnb = stp.tile([128, 1], F32, name=f"lnb{li}", tag="lnb")
            ps3 = psw.tile([128, 8], F32, name=f"ps3{li}", tag="pw")
            nc.tensor.matmul(ps3[:, 0:1], onr[0:1, :], lam_t[0:1, li:li + 1],
                             start=True, stop=True)
            nc.vector.tensor_copy(lnb[:, 0:1], ps3[:, 0:1])
            nc.vector.tensor_mul(lnb[:, 0:1], lnb[:, 0:1], lnb[:, 0:1])  # lamS^2
            for m in range(2):
                for fp in range(4):
                    fsl = slice(2 * fp, 2 * fp + 2)
                    sl2 = (slice(None), m, fsl, slice(None))
                    af_r = rp.tile([128, 2, N], BF16, name=f"af_r{li}{m}{fp}", tag="afr", bufs=1)
                    af_i = rp.tile([128, 2, N], BF16, name=f"af_i{li}{m}{fp}", tag="afi", bufs=1)
                    mg = rp.tile([128, 2, N], F32, name=f"mg{li}{m}{fp}", tag="mg", bufs=1)
                    mg2 = psm.tile([128, 512], F32, name=f"mg2{li}{m}{fp}", tag="psm")
                    if fp < 3:
                        nc.vector.tensor_sub(af_r[:, :, :], u2r[:, m, 2 * fp + 1:2 * fp + 3, :], u2r[sl2])
                        nc.vector.tensor_sub(af_i[:, :, :], u2i[:, m, 2 * fp + 1:2 * fp + 3, :], u2i[sl2])
                        nc.vector.scalar_tensor_tensor(af_r[:, :, :], af_r[:, :, :], SIGMA,
                                                       y2r[sl2], op0=ALU.mult, op1=ALU.add)
                        nc.vector.scalar_tensor_tensor(af_i[:, :, :], af_i[:, :, :], SIGMA,
                                                       y2i[sl2], op0=ALU.mult, op1=ALU.add)
                    else:
                        nc.vector.tensor_sub(af_r[:, 0, :], u2r[:, m, 7, :], u2r[:, m, 6, :])
                        nc.vector.tensor_sub(af_i[:, 0, :], u2i[:, m, 7, :], u2i[:, m, 6, :])
                        nc.vector.tensor_sub(af_r[:, 1, :], hu_r[:, m, :], u2r[:, m, 7, :])
                        nc.vector.tensor_sub(af_i[:, 1, :], hu_i[:, m, :], u2i[:, m, 7, :])
                        nc.vector.scalar_tensor_tensor(af_r[:, 0, :], af_r[:, 0, :], SIGMA,
                                                       y2r[:, m, 6, :], op0=ALU.mult, op1=ALU.add)
                        nc.vector.scalar_tensor_tensor(af_i[:, 0, :], af_i[:, 0, :], SIGMA,
                                                       y2i[:, m, 6, :], op0=ALU.mult, op1=ALU.add)
                        nc.vector.scalar_tensor_tensor(af_r[:, 1, :], af_r[:, 1, :], bc(3),
                                                       y2r[:, m, 7, :], op0=ALU.mult, op1=ALU.add)
                        nc.vector.scalar_tensor_tensor(af_i[:, 1, :], af_i[:, 1, :], bc(3),
                                                       y2i[:, m, 7, :], op0=ALU.mult, op1=ALU.add)
                    nc.vector.tensor_mul(mg[:, :, :], af_r[:, :, :], af_r[:, :, :])
                    nc.vector.tensor_mul(mg2[:, :].rearrange("p (a b) -> p a b", a=2),
                                         af_i[:, :, :], af_i[:, :, :])
                    nc.vector.tensor_add(mg[:, :, :].rearrange("p a b -> p (a b)"), mg[:, :, :].rearrange("p a b -> p (a b)"), mg2[:, :])
                    # mg = min(lamS / |argf2|, 1) = sqrt(min(lamS^2/mag2, 1))
                    nc.vector.reciprocal(mg[:, :, :], mg[:, :, :])
                    nc.vector.tensor_scalar_mul(mg[:, :, :], mg[:, :, :], lnb[:, 0:1])
                    nc.vector.tensor_scalar_min(mg[:, :, :], mg[:, :, :], 1.0)
                    nc.scalar.activation(mg[:, :, :], mg[:, :, :], AF.Sqrt)
                    nc.vector.tensor_mul(y2r[sl2], af_r[:, :, :], mg[:, :, :])
                    nc.vector.tensor_mul(y2i[sl2], af_i[:, :, :], mg[:, :, :])
            # y2 halo AG
            nc.sync.dma_start(out=hy_in[0, 0, :, :, :], in_=y2r[:, :, 7, :])
            nc.sync.dma_start(out=hy_in[0, 1, :, :, :], in_=y2i[:, :, 7, :])
            coll("AllGather", ALU.bypass, RG, [hy_in.ap()], [hy_out.ap()])
            nc.sync.dma_start(
                out=hy_r[:, :, :],
                in_=hy_out[bass.ds(idx_dn, 1), 0, :, :, :].rearrange("o p c y -> (o p) c y"))
            nc.sync.dma_start(
                out=hy_i[:, :, :],
                in_=hy_out[bass.ds(idx_dn, 1), 1, :, :, :].rearrange("o p c y -> (o p) c y"))

            # ---------- pixel side: deposit, argg1k, Gram ----------
            for p in range(2):
                eng_d = nc.gpsimd if p == 0 else nc.scalar
                for s_ in range(NCORE):
                    eng_d.dma_start(
                        out=y1ps[64 * p + 8 * s_:64 * p + 8 * s_ + 8, :],
                        in_=b1_out[s_, p, :, :, :].rearrange("f k y -> f (k y)"))
            nc.vector.scalar_tensor_tensor(y1ps[:, 0:4096], y1ps[:, 0:4096], -TAU,
                                           xps[:, 0:4096], op0=ALU.mult, op1=ALU.add)
            nc.vector.scalar_tensor_tensor(y1ps[:, 4096:8192], y1ps[:, 4096:8192],
                                           -TAU, xps[:, 4096:8192],
                                           op0=ALU.mult, op1=ALU.add)
            psG = psw.tile([128, 128], F32, name=f"psG{li}", tag="pw")
            tk_prev = None
            for k in range(64):
                ks = slice(128 * k, 128 * (k + 1))
                psT = psp.tile([128, 512], BF16, name=f"psT{li}_{k}", tag="ps")
                nc.tensor.transpose(psT[:, 0:128], y1ps[:, ks], C["I128b"][:, :])
                Tk = tkp.tile([128, 128], BF16, name=f"Tk{li}_{k}", tag="Tk")
                nc.scalar.copy(out=Tk[:, :], in_=psT[:, 0:128])
                if tk_prev is not None:
                    nc.tensor.matmul(psG[:, :], tk_prev[:, :], tk_prev[:, :],
                                     start=(k == 1), stop=False)
                tk_prev = Tk
            nc.tensor.matmul(psG[:, :], tk_prev[:, :], tk_prev[:, :],
                             start=False, stop=True)
            # combine blocks: Gr = RtR + ItI, Gi = RtI - ItR  (realign via DMA)
            sgf = stp.tile([128, 128], F32, name=f"sgf{li}", tag="sgf")
            sgx = stp.tile([64, 128], F32, name=f"sgx{li}", tag="sgx")
            nc.vector.tensor_copy(sgf[:, :], psG[:, :])
            nc.sync.dma_start(out=sgx[0:64, :], in_=sgf[64:128, :])
            gl = stp.tile([64, 192], F32, name=f"gl{li}", tag="gl")
            nc.vector.tensor_add(gl[:, 0:64], sgf[0:64, 0:64], sgx[:, 64:128])
            nc.vector.tensor_sub(gl[:, 64:128], sgf[0:64, 64:128], sgx[:, 0:64])
            nc.vector.tensor_sub(gl[:, 128:192], sgx[:, 0:64], sgf[0:64, 64:128])
            nc.sync.dma_start(out=g_in[:, :], in_=gl[:, :])
            coll("AllReduce", ALU.add, RG, [g_in.ap()], [g_out.ap()])

            # ---------- inverse fft of y1 (emitted interleaved with W chain) ----
            Qt = bigp.tile([128, 2, NFL, 512], BF16, name=f"Qt{li}", tag="big", bufs=1)

            inv_units = []
            for f in range(NFL):
                inv_units.append(lambda f=f: fwd_stageA(Qt, y1r, y1i, C["SI0"], C["SI1"],
                                                        f"i{li}", frames=[f]))

            def inv_stageB(m, fp, li=li, Qt=Qt):
                fsl = slice(2 * fp, 2 * fp + 2)
                sl2 = (slice(None), m, fsl, slice(None))
                pr = psp.tile([128, 512], F32, name=f"pm_r{li}{m}{fp}", tag="ps")
                pi = psp.tile([128, 512], F32, name=f"pm_i{li}{m}{fp}", tag="ps")
                stageB(pr[:, :], pi[:, :], Qt, m, fp, inv=True)
                for ppp, y2t, x2t, hyt in ((pr, y2r, x2r, hy_r), (pi, y2i, x2i, hy_i)):
                    nc.tensor.matmul(ppp[:, :], C["I1b"][:, :], x2t[sl2],
                                     start=False, stop=False)
                    if fp == 3:
                        nc.tensor.matmul(ppp[:, 0:256], C["ITb"][:, :],
                                         y2t[:, m, 6, :], start=False, stop=False)
                        nc.tensor.matmul(ppp[:, 256:512], IT7[:, :],
                                         y2t[:, m, 7, :], start=False, stop=False)
                    else:
                        nc.tensor.matmul(ppp[:, :], C["ITb"][:, :], y2t[sl2],
                                         start=False, stop=False)
                    if fp == 0:
                        nc.tensor.matmul(ppp[:, 0:256], ITn0[:, :], hyt[:, m, :],
                                         start=False, stop=False)
                        nc.tensor.matmul(ppp[:, 256:512], C["ITnb"][:, :],
                                         y2t[:, m, 0, :], start=False, stop=True)
                    else:
                        nc.tensor.matmul(ppp[:, :], C["ITnb"][:, :],
                                         y2t[:, m, 2 * fp - 1:2 * fp + 1, :],
                                         start=False, stop=True)
                if not last:
                    nc.vector.scalar_tensor_tensor(u2r[sl2], x2r[sl2], -0.5, pr[:, :],
                                                   op0=ALU.mult, op1=ALU.add)
                    nc.vector.tensor_scalar_mul(u2r[sl2], u2r[sl2], 2.0)
                    nc.vector.scalar_tensor_tensor(u2i[sl2], x2i[sl2], -0.5, pi[:, :],
                                                   op0=ALU.mult, op1=ALU.add)
                    nc.vector.tensor_scalar_mul(u2i[sl2], u2i[sl2], 2.0)
                nc.vector.tensor_copy(x2r[sl2], pr[:, :])
                nc.vector.tensor_copy(x2i[sl2], pi[:, :])

            for m in range(2):
                for fp in [1, 2, 3, 0]:
                    inv_units.append(lambda m=m, fp=fp: inv_stageB(m, fp))
            inv_iter = iter(inv_units)

            def drip(k=1):
                for _ in range(k):
                    u = next(inv_iter, None)
                    if u is not None:
                        u()

            def drain():
                for u in inv_iter:
                    u()

            # ---------- W chain (S-form) ----------
            Sg = smat("Sg")
            nc.sync.dma_start(out=Sg[0:64, 0:64], in_=g_out[:, 0:64])
            nc.sync.dma_start(out=Sg[64:128, 64:128], in_=g_out[:, 0:64])
            nc.sync.dma_start(out=Sg[64:128, 0:64], in_=g_out[:, 64:128])
            nc.sync.dma_start(out=Sg[0:64, 64:128], in_=g_out[:, 128:192])
            drip(2)

            scrD = stp.tile([128, 128], F32, name=f"scrD{li}", tag="scrD")
            trs = stp.tile([128, 1], F32, name=f"trs{li}", tag="trs")
            lg = stp.tile([1, 8], F32, name=f"lg{li}", tag="lg")
            rtr = stp.tile([1, 1], F32, name=f"rtr{li}", tag="rtr")
            bres = stp.tile([128, 1], F32, name=f"bres{li}", tag="bres")
            psb = psw.tile([128, 8], F32, name=f"psb{li}", tag="pw")
            pst = psw.tile([1, 8], F32, name=f"pst{li}", tag="pw")

            def trace_of(src, dstcol):
                # per-partition diag pick + accumulate, then cross-partition sum
                nc.vector.scalar_tensor_tensor(scrD[:, :], src, 1.0,
                                               C["I128f"][:, :], op0=ALU.mult,
                                               op1=ALU.mult, accum_out=trs[:, 0:1])
                nc.tensor.matmul(pst[0:1, dstcol:dstcol + 1], trs[:, :], on1[:, :],
                                 start=True, stop=True)
                nc.vector.tensor_copy(lg[0:1, dstcol:dstcol + 1],
                                      pst[0:1, dstcol:dstcol + 1])

            def bcast(src_ap, col):
                nc.tensor.matmul(psb[:, col:col + 1], onr[0:1, :], src_ap,
                                 start=True, stop=True)
                nc.vector.tensor_copy(bres[:, 0:1], psb[:, col:col + 1])

            # --- norm estimation: squarings with grouped renorm ---
            Bm = smat("Bm")
            Bp = smat("Bp")
            trace_of(Sg[:, :], 0)                      # lg0 = trS(G)
            nc.vector.reciprocal(rtr[0:1, :], lg[0:1, 0:1])
            bcast(rtr[0:1, :], 0)
            nc.vector.tensor_scalar_mul(Bm[:, :], Sg[:, :], bres[:, 0:1])
            drip(1)
            ktot = 0
            grp = 0
            while ktot < N_SQ:
                steps = min(3, N_SQ - ktot)
                for _ in range(steps):
                    psS = psw.tile([128, 128], F32, name=f"psS{li}_{ktot}", tag="pw")
                    nc.tensor.matmul(psS[:, :], Bm[:, :], Bm[:, :], start=True, stop=True)
                    nc.scalar.copy(out=Bp[:, :], in_=psS[:, :])
                    Bm, Bp = Bp, Bm
                    ktot += 1
                    drip(1)
                trace_of(Bm[:, :], 2 + grp)            # lg[2+grp] = trace of group
                if ktot < N_SQ:
                    nc.vector.reciprocal(rtr[0:1, :], lg[0:1, 2 + grp:3 + grp])
                    bcast(rtr[0:1, :], 1)
                    nc.vector.tensor_scalar_mul(Bm[:, :], Bm[:, :], bres[:, 0:1])
                grp += 1
                drip(1)
            # lam = trS0 * exp((32*ln t3 + 4*ln t6 + ln t8 - ln 2)/256)
            nc.scalar.activation(lg[0:1, 5:6], lg[0:1, 2:3], AF.Ln)
            nc.scalar.activation(lg[0:1, 6:7], lg[0:1, 3:4], AF.Ln)
            nc.scalar.activation(lg[0:1, 7:8], lg[0:1, 4:5], AF.Ln)
            nc.vector.tensor_scalar(lg[0:1, 5:6], lg[0:1, 5:6], 32.0, None, op0=ALU.mult)
            nc.vector.tensor_scalar(lg[0:1, 6:7], lg[0:1, 6:7], 4.0, None, op0=ALU.mult)
            nc.vector.tensor_add(lg[0:1, 5:6], lg[0:1, 5:6], lg[0:1, 6:7])
            nc.vector.tensor_add(lg[0:1, 5:6], lg[0:1, 5:6], lg[0:1, 7:8])
            nc.vector.tensor_scalar(lg[0:1, 5:6], lg[0:1, 5:6], 1.0,
                                    -float(np.log(2.0)), op0=ALU.mult, op1=ALU.add)
            nc.scalar.activation(lg[0:1, 6:7], lg[0:1, 5:6], AF.Exp,
                                 scale=1.0 / (2.0 ** N_SQ))
            nc.vector.tensor_mul(lg[0:1, 6:7], lg[0:1, 6:7], lg[0:1, 0:1])  # lam
            nc.vector.reciprocal(rtr[0:1, :], lg[0:1, 6:7])
            bcast(rtr[0:1, :], 2)
            Gh = smat("Gh")
            nc.vector.tensor_scalar_mul(Gh[:, :], Sg[:, :], bres[:, 0:1])
            drip(1)

            # --- X = (Gh - t2*I)/(1-t2) ---
            t2s = stp.tile([1, 1], F32, name=f"t2s{li}", tag="t2s")
            nc.scalar.activation(t2s[0:1, :], lam_t[0:1, NLAYERS + li:NLAYERS + li + 1],
                                 AF.Square, scale=TAU)
            ths = stp.tile([1, 1], F32, name=f"ths{li}", tag="ths")
            nc.vector.tensor_scalar_mul(ths[0:1, 0:1],
                                        lam_t[0:1, NLAYERS + li:NLAYERS + li + 1], TAU)
            onem = stp.tile([1, 1], F32, name=f"onem{li}", tag="onem")
            nc.vector.tensor_scalar(onem[0:1, :], t2s[0:1, :], -1.0, 1.0,
                                    op0=ALU.mult, op1=ALU.add)
            nc.vector.reciprocal(onem[0:1, :], onem[0:1, :])
            nt2 = stp.tile([128, 1], F32, name=f"nt2{li}", tag="nt2")
            bcast(t2s[0:1, :], 3)
            nc.vector.tensor_scalar_mul(nt2[:, 0:1], bres[:, 0:1], -1.0)
            i1m = stp.tile([128, 1], F32, name=f"i1m{li}", tag="i1m")
            bcast(onem[0:1, :], 4)
            nc.vector.tensor_copy(i1m[:, 0:1], bres[:, 0:1])
            X = smat("X")
            nc.vector.scalar_tensor_tensor(X[:, :], C["I128f"][:, :], nt2[:, 0:1],
                                           Gh[:, :], op0=ALU.mult, op1=ALU.add)
            nc.vector.tensor_scalar_mul(X[:, :], X[:, :], i1m[:, 0:1])
            drip(1)

            # --- sign chain + chebyshev, interleaved (independent given Gh) ---
            T = smat("T")
            nc.vector.tensor_scalar_mul(T[:, :], Gh[:, :], 2.0 / Q_HI)
            nc.vector.scalar_tensor_tensor(T[:, :], C["I128f"][:, :], -1.0,
                                           T[:, :], op0=ALU.mult, op1=ALU.add)
            cb1 = smat("cb1")
            cb2 = smat("cb2")
            cbt = smat("cbt")
            nc.vector.memset(cb1[:, :], 0.0)
            nc.vector.memset(cb2[:, :], 0.0)
            mats = [cb1, cb2, cbt]
            X2 = smat("X2")
            Yp = smat("Yp")

            def sign_step(k_):
                a_, b_, c_ = SIGN_COEFFS[k_]
                ps_a = psw.tile([128, 128], F32, name=f"s2_{li}_{k_}", tag="pw")
                nc.tensor.matmul(ps_a[:, :], X[:, :], X[:, :], start=True, stop=True)
                nc.scalar.copy(out=X2[:, :], in_=ps_a[:, :])
                ps_b = psw.tile([128, 128], F32, name=f"s4_{li}_{k_}", tag="pw")
                nc.tensor.matmul(ps_b[:, :], X2[:, :], X2[:, :], start=True, stop=True)
                nc.vector.scalar_tensor_tensor(Yp[:, :], X2[:, :], b_ / c_,
                                               ps_b[:, :], op0=ALU.mult, op1=ALU.add)
                nc.vector.scalar_tensor_tensor(Yp[:, :], C["I128f"][:, :], a_ / c_,
                                               Yp[:, :], op0=ALU.mult, op1=ALU.add)
                ps_c = psw.tile([128, 128], F32, name=f"sx_{li}_{k_}", tag="pw")
                nc.tensor.matmul(ps_c[:, :], X[:, :], Yp[:, :], start=True, stop=True)
                sc_ = c_ * 0.5 if k_ == len(SIGN_COEFFS) - 1 else c_
                nc.scalar.activation(X[:, :], ps_c[:, :], AF.Copy, scale=sc_)

            def cheb_step(ci_idx, mats=mats):
                ci = Q_COEF[::-1][:-1][ci_idx]
                bb1, bb2, tt = mats
                ps_t = psw.tile([128, 128], F32, name=f"cl{li}_{ci_idx}", tag="pw")
                nc.tensor.matmul(ps_t[:, :], T[:, :], bb1[:, :], start=True, stop=True)
                nc.vector.scalar_tensor_tensor(tt[:, :], ps_t[:, :], 2.0,
                                               bb2[:, :], op0=ALU.mult,
                                               op1=ALU.subtract)
                nc.vector.scalar_tensor_tensor(tt[:, :], C["I128f"][:, :], ci,
                                               tt[:, :], op0=ALU.mult, op1=ALU.add)
                mats[:] = [tt, bb1, bb2]

            n_cheb = len(Q_COEF) - 1
            for k_ in range(max(len(SIGN_COEFFS), n_cheb)):
                if k_ < len(SIGN_COEFFS):
                    sign_step(k_)
                if k_ < n_cheb:
                    cheb_step(k_)
                drip(1)
            # P = 0.5 I + X(=0.5 sign)
            P = smat("P")
            nc.vector.scalar_tensor_tensor(P[:, :], C["I128f"][:, :], 0.5,
                                           X[:, :], op0=ALU.mult, op1=ALU.add)
            bb1, bb2, _ = mats
            Q = smat("Q")
            ps_q = psw.tile([128, 128], F32, name=f"qf{li}", tag="pw")
            nc.tensor.matmul(ps_q[:, :], T[:, :], bb1[:, :], start=True, stop=True)
            nc.vector.scalar_tensor_tensor(Q[:, :], ps_q[:, :], 1.0,
                                           bb2[:, :], op0=ALU.mult, op1=ALU.subtract)
            nc.vector.scalar_tensor_tensor(Q[:, :], C["I128f"][:, :], Q_COEF[0],
                                           Q[:, :], op0=ALU.mult, op1=ALU.add)
            # W = P - ths * P @ Q
            ps_w = psw.tile([128, 128], F32, name=f"pq{li}", tag="pw")
            nc.tensor.matmul(ps_w[:, :], P[:, :], Q[:, :], start=True, stop=True)
            nth = stp.tile([128, 1], F32, name=f"nth{li}", tag="nth")
            bcast(ths[0:1, 0:1], 5)
            nc.vector.tensor_scalar_mul(nth[:, 0:1], bres[:, 0:1], -1.0)
            Wt = smat("Wt")
            nc.vector.scalar_tensor_tensor(Wt[:, :], ps_w[:, :], nth[:, 0:1],
                                           P[:, :], op0=ALU.mult, op1=ALU.add)
            # recon stationary: S(conj W) = S(W) with off-diagonal blocks negated
            Wsb = wp.tile([128, 128], BF16, name=f"Wsb{li}", tag="Wsb")
            nc.vector.tensor_copy(Wsb[:, :], Wt[:, :])
            nc.vector.tensor_scalar_mul(Wsb[0:64, 64:128], Wt[0:64, 64:128], -1.0)
            nc.vector.tensor_scalar_mul(Wsb[64:128, 0:64], Wt[64:128, 0:64], -1.0)
            drain()

            # ---------- recon + V + A2A#2 (stacked) ----------
            for ch in range(16):
                cs = slice(512 * ch, 512 * (ch + 1))
                pX = psp.tile([128, 512], F32, name=f"pX{li}_{ch}", tag="ps")
                nc.tensor.matmul(pX[:, :], Wsb[:, :], y1ps[:, cs],
                                 start=True, stop=True)
                if not last:
                    nc.vector.scalar_tensor_tensor(y1ps[:, cs], xps[:, cs], -0.5,
                                                   pX[:, :], op0=ALU.mult, op1=ALU.add)
                    nc.vector.tensor_copy(xps[:, cs], pX[:, :])
                    nc.vector.tensor_scalar_mul(y1ps[:, cs], y1ps[:, cs], 2.0 * C2)
                else:
                    nc.vector.tensor_copy(y1ps[:, cs], pX[:, :])
            for dst in range(NCORE):
                for p in range(2):
                    eng_s = nc.gpsimd if p == 0 else nc.scalar
                    eng_s.dma_start(
                        out=b2_in[dst, p, :, :, :].rearrange("f k y -> f (k y)"),
                        in_=y1ps[64 * p + 8 * dst:64 * p + 8 * dst + 8, :])
            coll("AllToAll", ALU.bypass, RG, [b2_in.ap()], [b2_out.ap()])

            # load V for next layer's y1 update into resident tiles
            if not last:
                for p, vt in ((0, vkr), (1, vki)):
                    eng_v = nc.gpsimd if p == 0 else nc.scalar
                    for s_ in range(NCORE):
                        m, q_ = s_ // 4, s_ % 4
                        eng_v.dma_start(
                            out=vt[32 * q_:32 * (q_ + 1), m, :, :],
                            in_=b2_out[s_, p, :, :, :]
                                .rearrange("f k y -> k f y"))
                push_u2_halo()

        # ---------------- final ----------------
        xfr = bigp.tile([128, 2, NFL, N], BF16, name="xfr", tag="bigs", bufs=2)
        xfi = bigp.tile([128, 2, NFL, N], BF16, name="xfi", tag="bigs", bufs=2)
        for p, xt in ((0, xfr), (1, xfi)):
            for m in range(2):
                for sq_ in range(4):
                    nc.sync.dma_start(
                        out=xt[32 * sq_:32 * (sq_ + 1), m, :, :],
                        in_=b2_out[4 * m + sq_, p, :, :, :]
                            .rearrange("f k y -> k f y"))
        Qtf = bigp.tile([128, 2, NFL, 512], BF16, name="Qtf", tag="big", bufs=1)
        fwd_stageA(Qtf, xfr, xfi, C["SI0"], C["SI1"], "fin")
        for m in range(2):
            for fp in range(4):
                pr = psp.tile([128, 512], F32, name=f"pf_r{m}{fp}", tag="ps")
                pi = psp.tile([128, 512], F32, name=f"pf_i{m}{fp}", tag="ps")
                stageB(pr[:, :], pi[:, :], Qtf, m, fp, inv=True, stop=True)
                sl2 = (slice(None), m, slice(2 * fp, 2 * fp + 2), slice(None))
                op_r = bigp.tile([128, 2, N], F32, name=f"op_r{m}{fp}", tag="bigs", bufs=2)
                op_i = bigp.tile([128, 2, N], F32, name=f"op_i{m}{fp}", tag="bigs", bufs=2)
                nc.vector.scalar_tensor_tensor(op_r[:, :, :], pr[:, :], -1.0 / TAU,
                                               x2r[sl2], op0=ALU.mult, op1=ALU.add)
                nc.vector.scalar_tensor_tensor(op_i[:, :, :], pi[:, :], -1.0 / TAU,
                                               x2i[sl2], op0=ALU.mult, op1=ALU.add)
                nc.sync.dma_start(out=out_r[:, m, 2 * fp:2 * fp + 2, :], in_=op_r[:, :, :])
                nc.sync.dma_start(out=out_i[:, m, 2 * fp:2 * fp + 2, :], in_=op_i[:, :, :])

        stack.close()

    nc.compile()
    return nc


_CACHE = {}


def _get_nc(n_layers=NLAYERS):
    if n_layers not in _CACHE:
        _CACHE[n_layers] = build(n_layers)
    return _CACHE[n_layers]


def host_shard(d_real, d_imag, lambdaS, lambdaL):
    d_r = np.asarray(d_real, np.float32).reshape(NF, N, N)
    d_i = np.asarray(d_imag, np.float32).reshape(NF, N, N)
    dTr = d_r.transpose(0, 2, 1)
    dTi = d_i.transpose(0, 2, 1)
    in_maps = []
    for c in range(NCORE):
        fr = slice(8 * c, 8 * c + 8)
        dk_rc = dTr[fr].reshape(NFL, 2, 128, N).transpose(2, 1, 0, 3).copy()
        dk_ic = dTi[fr].reshape(NFL, 2, 128, N).transpose(2, 1, 0, 3).copy()
        blk_r = dTr[:, 32 * c:32 * c + 32, :].reshape(NF, 8192)
        blk_i = dTi[:, 32 * c:32 * c + 32, :].reshape(NF, 8192)
        dps_c = np.ascontiguousarray(np.concatenate([blk_r, blk_i], axis=0))
        m0 = 0.0 if c == 0 else 1.0
        m7 = 0.0 if c == NCORE - 1 else 1.0
        in_maps.append({
            "dk_r": dk_rc, "dk_i": dk_ic, "dps": dps_c,
            "lamS": np.asarray(lambdaS, np.float32).reshape(1, NLAYERS).copy(),
            "lamL": np.asarray(lambdaL, np.float32).reshape(1, NLAYERS).copy(),
            "cmeta": np.array([[0, 0, m0, m7]], np.float32),
            "cmeta_u": np.array([[min(c + 1, NCORE - 1), max(c - 1, 0)]], np.uint32),
        })
    return in_maps


def host_gather(results):
    out = np.zeros((NF, N, N), np.complex64)
    for c, res in enumerate(results):
        img = (res["out_r"] + 1j * res["out_i"]).astype(np.complex64)
        out[8 * c:8 * c + 8] = img.transpose(2, 1, 0, 3).reshape(NFL, N, N)
    return out.reshape(1, 1, NF, N, N)


def kernel(d_real, d_imag, lambdaS, lambdaL):
    nc = _get_nc()
    in_maps = host_shard(d_real, d_imag, lambdaS, lambdaL)
    res = bass_utils.run_bass_kernel_spmd(nc, in_maps, core_ids=list(range(NCORE)))
    return host_gather(res.results)
